# revision 18
# baseline (speedup 1.0000x reference)
"""Trainium2 Bass kernel for nn_Node3DEmbeddingv2 (gnn_message_passing).

Strategy (8 NeuronCores, SPMD, data-parallel over flattened (batch, query-row)):
  - 1536 query rows split into 8 x 192 (4 cores per batch).
  - Phase A (per 96-row strip): d^2 = |pi|^2 - 2 pi.pj + |pj|^2 via one fp16
    matmul against host-prepped 3-component fp16 splits of pos (24
    contraction rows, exact to f32), ACT Relu(x - 5e-4) to clamp the
    cancellation noise on the diagonal to exactly 0, ACT Sqrt, then an exact
    3-way fp16 split of d (33 mantissa bits).
  - Phase B (per 24-row block): flatten the fp16 d-components into a
    [3, 24*768] moving tile (partition trio 0-2), broadcast each row's 768
    distances across all 128 gaussian-channel partitions with a [3,128]-ones
    fp16 matmul per 512-col PSUM window (one stationary, reloaded cheaply /
    elided by ldw-opt). One ScalarE op per [128,1536] PSUM unit computes the
    Gaussian:
      Derivative_Erf(scale_k * d + bias_k) = 2/sqrt(pi) * exp(-((d-m_k)/s_k)^2/2)
    writing fp16. The key-axis sum runs as an in-place halving add-tree on
    DVE (fp16 tensor_tensor = 2x perf mode; tensor_reduce is 1x-capped) down
    to width 6, then one tiny f32 tensor_reduce into S.
  - Phase C: channel constants on the summed [128,192] tensor, fp16
    feature_proj MLP (gelu between two matmuls), DMA-transpose (fp16) back
    to row-major, add the host-computed angle/time tail, DMA out [192,512].
  - Host (numpy, negligible): angle MLP, sinusoidal time embedding MLP,
    masking, per-core input prep; all heavy compute is on-device.
"""

import math

import numpy as np

# Problem constants (hardcoded per the task contract).
B, N, K, E = 2, 768, 128, 512
INTER = E // 2
NCORES = 8
RPC = (B * N) // NCORES  # 192 rows per core
PI_REF = 3.14159         # matches reference's gaussian constant

NBLOCK = 8               # 24-row phase-B blocks per core
BR = 24                  # rows per block
NROWS_A = 24             # contraction rows of the d^2 matmul
D2_SHIFT = 5.0e-4        # relu clamp: zeroes |d| < 0.022 (true data min ~0.5)

# Derivative_Erf table semantics: d/dx erf(x) = 2/sqrt(pi) * exp(-x^2).
# DERF_INV converts the table output back to exp(-x^2).
DERF_INV = math.sqrt(math.pi) / 2.0

USE_LDW_OPT = False  # walrus rejects ldw-opt for these ldweights forms

_COMPILED = {}


def _enable_ldw_opt():
    """Flip walrus's redundant-LDWEIGHTS elimination on: the 288 broadcast
    matmuls reuse one stationary [3,128] ones matrix and the per-matmul
    reload serializes ~124ns each on the PE. Only safe with zero f32
    matmuls in the module (this kernel is all-fp16). Correctness is
    re-verified end-to-end against the oracle after any flag change."""
    from concourse import bass_utils

    if getattr(bass_utils, "_ldw_opt_patched", False):
        return
    orig_run = bass_utils.run_command

    def run_patched(argv, **kw):
        argv = [
            a.replace("--enable-ldw-opt=false", "--enable-ldw-opt=true")
            if isinstance(a, str) else a
            for a in argv
        ]
        return orig_run(argv, **kw)

    bass_utils.run_command = run_patched
    bass_utils._ldw_opt_patched = True


def _build_nc():
    import concourse.bass as bass
    import concourse.bacc as bacc
    from concourse import mybir
    from concourse.tile import TileContext

    if USE_LDW_OPT:
        _enable_ldw_opt()

    f32 = mybir.dt.float32
    f16 = mybir.dt.float16
    AF = mybir.ActivationFunctionType

    nc = bacc.Bacc("TRN2", target_bir_lowering=False)

    mkeys = nc.dram_tensor("mkeys", [NROWS_A, N], f16, kind="ExternalInput")
    squery = nc.dram_tensor("squery", [NROWS_A, RPC], f16, kind="ExternalInput")
    esc = nc.dram_tensor("esc", [K, 1], f32, kind="ExternalInput")
    ebi = nc.dram_tensor("ebi", [K, 1], f32, kind="ExternalInput")
    postc = nc.dram_tensor("postc", [K, 1], f32, kind="ExternalInput")
    w1 = nc.dram_tensor("w1", [K, K], f16, kind="ExternalInput")
    w2 = nc.dram_tensor("w2", [K, INTER], f16, kind="ExternalInput")
    ident = nc.dram_tensor("ident", [128, 128], f32, kind="ExternalInput")
    rest = nc.dram_tensor("rest", [RPC, E], f32, kind="ExternalInput")
    out = nc.dram_tensor("out", [RPC, E], f32, kind="ExternalOutput")

    with TileContext(nc) as tc:
        with nc.allow_low_precision(reason="fp16 gaussian accumulate, verified vs oracle"), \
             tc.tile_pool(name="sb", bufs=1) as sb:
            # ---- constant loads (spread over queues; phase A only needs
            # mkeys/squery, which go first on their own queues) ----
            mk_sb = sb.tile([NROWS_A, N], f16, tag="mk_sb")
            nc.sync.dma_start(out=mk_sb, in_=mkeys[:, :])
            sq_sb = sb.tile([NROWS_A, RPC], f16, tag="sq_sb")
            nc.scalar.dma_start(out=sq_sb, in_=squery[:, :])
            esc_sb = sb.tile([K, 1], f32, tag="esc_sb")
            nc.scalar.dma_start(out=esc_sb, in_=esc[:, :])
            ebi_sb = sb.tile([K, 1], f32, tag="ebi_sb")
            nc.scalar.dma_start(out=ebi_sb, in_=ebi[:, :])
            postc_sb = sb.tile([K, 1], f32, tag="postc_sb")
            nc.sync.dma_start(out=postc_sb, in_=postc[:, :])
            w1_sb = sb.tile([K, K], f16, tag="w1_sb")
            nc.sync.dma_start(out=w1_sb, in_=w1[:, :])
            w2_sb = sb.tile([K, INTER], f16, tag="w2_sb")
            nc.sync.dma_start(out=w2_sb, in_=w2[:, :])
            id_sb = sb.tile([128, 128], f32, tag="id_sb")
            nc.sync.dma_start(out=id_sb, in_=ident[:, :])
            rest_sb = sb.tile([96, 2, E], f32, tag="rest_sb")
            nc.sync.dma_start(
                out=rest_sb, in_=rest.rearrange("(s p) e -> p s e", s=2)
            )
            ones3 = sb.tile([3, 128], f16, tag="ones3")
            nc.vector.memset(ones3, 1.0)
            shift_sb = sb.tile([96, 1], f32, tag="shift_sb")
            nc.vector.memset(shift_sb, -D2_SHIFT)

            S = sb.tile([K, RPC], f32, tag="S")

            dcomps = {}

            def phase_a(st):
                # d^2 matmul + relu/sqrt + fp16 splits for one 96-row strip
                d2p = psA.tile([96, N], f32, tag="d2p", bufs=1)
                cols = slice(96 * st, 96 * (st + 1))
                nc.tensor.matmul(
                    d2p[:, 0:512], sq_sb[:, cols], mk_sb[:, 0:512],
                    start=True, stop=True,
                )
                nc.tensor.matmul(
                    d2p[:, 512:N], sq_sb[:, cols], mk_sb[:, 512:N],
                    start=True, stop=True,
                )
                dr = sb.tile([96, N], f32, tag="dr", bufs=2)
                nc.scalar.activation(dr, d2p, AF.Relu, bias=shift_sb)
                d32 = sb.tile([96, N], f32, tag="d32", bufs=2)
                nc.scalar.sqrt(d32, dr)
                dc = sb.tile([96, 3, N], f16, tag="dcomp", bufs=2)
                nc.vector.tensor_copy(dc[:, 0, :], d32)
                r1 = sb.tile([96, N], f32, tag="r1", bufs=2)
                nc.vector.tensor_sub(r1, d32, dc[:, 0, :])
                nc.vector.tensor_copy(dc[:, 1, :], r1)
                r2 = sb.tile([96, N], f32, tag="r2", bufs=2)
                nc.vector.tensor_sub(r2, r1, dc[:, 1, :])
                nc.vector.tensor_copy(dc[:, 2, :], r2)
                dcomps[st] = dc

            def bcast_matmul(out_ap, rhs, ldw):
                # nc.tensor.matmul with an explicit ldweights flag: all
                # broadcast matmuls share the ones3 stationary, so runs after
                # the first skip the per-matmul LDWEIGHTS reload (ldw=False).
                ifmap_ap = nc.tensor.lower_ap(rhs.opt({0}), opt=False)
                weights_ap = nc.tensor.lower_ap(
                    ones3.opt({0}), opt=False, for_matmul_weights=True
                )
                out_l = nc.tensor.lower_ap(out_ap)
                nc.tensor.add_instruction(
                    mybir.InstMatmult(
                        name=nc.get_next_instruction_name(),
                        replication_resolution=0,
                        replication_shift_amnt=0,
                        replication_num_rows=0,
                        start_tensor_calc=True,
                        stop_tensor_calc=True,
                        ins=[ifmap_ap, weights_ap],
                        outs=[out_l],
                        tile_position=(0, 0),
                        tile_size=(32, 128),
                        ldweights=ldw,
                    )
                )

            def phase_b(b):
                # broadcast + gaussian + key-sum for one 24-row block
                dc = dcomps[b // 4]
                p0 = BR * (b % 4)
                mfl = sb.tile([3, BR * N], f16, tag="mflat", bufs=2)
                for comp in range(3):
                    nc.gpsimd.dma_start(
                        out=mfl[comp : comp + 1, :].rearrange(
                            "p (i j) -> p i j", i=BR
                        ),
                        in_=dc[p0 : p0 + BR, comp, :],
                    )
                gsc = sb.tile([K, BR, N], f16, tag="gsc", bufs=2)
                for u in range(12):
                    pu = psB.tile([K, 1536], f32, tag="pu", bufs=2)
                    for w in range(3):
                        lo = 1536 * u + 512 * w
                        # reload weights on the first matmul after any
                        # other-stationary matmul ran on the PE (phase A of
                        # strip 1 is emitted between blocks 1 and 2)
                        ldw = u == 0 and w == 0 and b in (0, 2)
                        bcast_matmul(
                            pu[:, 512 * w : 512 * w + 512],
                            mfl[:, lo : lo + 512],
                            ldw,
                        )
                    nc.scalar.activation(
                        out=gsc[:, 2 * u : 2 * u + 2, :],
                        in_=pu,
                        func=AF.Derivative_Erf,
                        bias=ebi_sb,
                        scale=esc_sb,
                    )
                # in-place fp16 halving add-tree over the key axis (runs
                # during the NEXT block's gaussians — other gsc buffer)
                wdt = N // 2
                while wdt >= 6:
                    nc.vector.tensor_add(
                        gsc[:, :, 0:wdt], gsc[:, :, 0:wdt],
                        gsc[:, :, wdt : 2 * wdt],
                    )
                    wdt //= 2
                nc.vector.tensor_reduce(
                    out=S[:, BR * b : BR * (b + 1)],
                    in_=gsc[:, :, 0:6],
                    axis=mybir.AxisListType.X,
                    op=mybir.AluOpType.add,
                )

            with tc.tile_pool(name="psA", bufs=1, space="PSUM") as psA, \
                 tc.tile_pool(name="psB", bufs=1, space="PSUM") as psB:
                phase_a(0)
                for b in range(NBLOCK):
                    if b == 2:
                        phase_a(1)
                    phase_b(b)

            # ---- phase C: channel constants + feature_proj MLP + output ----
            with tc.tile_pool(name="psC", bufs=1, space="PSUM") as psC:
                for st in range(2):
                    rows = slice(96 * st, 96 * (st + 1))
                    nc.vector.tensor_scalar_mul(S[:, rows], S[:, rows], postc_sb)
                    s16 = sb.tile([K, 96], f16, tag="s16", bufs=2)
                    nc.vector.tensor_copy(s16, S[:, rows])
                    psum_h = psC.tile([K, 96], f32, tag="mlp_h", bufs=2)
                    nc.tensor.matmul(psum_h, w1_sb, s16, start=True, stop=True)
                    h16 = sb.tile([K, 96], f16, tag="h16", bufs=2)
                    nc.scalar.activation(h16, psum_h, AF.Gelu)
                    o_sb = sb.tile([K, 2, 96], f32, tag="o_sb", bufs=2)
                    for e in range(2):
                        psum_o = psC.tile([K, 96], f32, tag="mlp_o", bufs=2)
                        nc.tensor.matmul(
                            psum_o, w2_sb[:, 128 * e : 128 * (e + 1)], h16,
                            start=True, stop=True,
                        )
                        nc.scalar.copy(o_sb[:, e, :], psum_o)
                    for e in range(2):
                        psum_t = psC.tile([96, 128], f32, tag="tr", bufs=2)
                        nc.tensor.transpose(psum_t, o_sb[:, e, :], id_sb)
                        nc.vector.tensor_add(
                            rest_sb[:, st, 128 * e : 128 * (e + 1)],
                            rest_sb[:, st, 128 * e : 128 * (e + 1)],
                            psum_t,
                        )
                    nc.sync.dma_start(out=out[rows, :], in_=rest_sb[:, st, :])

    nc.compile()
    return nc


# ---------------- host-side prep (numpy) ----------------

def _erf_np(x):
    try:
        from scipy.special import erf
        return erf(x).astype(np.float32)
    except ImportError:
        f = np.frompyfunc(math.erf, 1, 1)
        return f(x.astype(np.float64)).astype(np.float32)


def _gelu_np(x):
    x = x.astype(np.float32)
    return (x * 0.5 * (1.0 + _erf_np(x / np.float32(math.sqrt(2.0))))).astype(
        np.float32
    )


def _silu_np(x):
    x = x.astype(np.float32)
    return (x / (1.0 + np.exp(-x))).astype(np.float32)


def _timestep_emb_np(t, dim):
    half = dim // 2
    freqs = np.exp(
        -np.log(10000.0) * np.arange(half, dtype=np.float32) / np.float32(half)
    ).astype(np.float32)
    a = t.astype(np.float32)[:, None] * freqs[None, :]
    return np.concatenate([np.sin(a), np.cos(a)], axis=-1).astype(np.float32)


def _host_tails(angle, mask_pos, time_pos, ang_w1, ang_w2, t_w1, t_b1, t_w2, t_b2):
    """rest[b, n, :] with rest[..., :INTER] = time_emb[..., :INTER] and
    rest[..., INTER:] = ang_f + time_emb[..., INTER:]."""
    angle = np.asarray(angle, np.float32)
    ang = np.where(np.isposinf(angle), np.float32(0.0), angle).astype(np.float32)
    ang_f = _gelu_np(ang @ np.asarray(ang_w1, np.float32)) @ np.asarray(
        ang_w2, np.float32
    )  # [B, N, INTER]

    def time_mlp(t):
        e = _timestep_emb_np(t, E)
        h = _silu_np(e @ np.asarray(t_w1, np.float32) + np.asarray(t_b1, np.float32))
        return (h @ np.asarray(t_w2, np.float32) + np.asarray(t_b2, np.float32)).astype(
            np.float32
        )

    tp = np.asarray(time_pos)
    te = time_mlp(tp)[:, None, :]                 # [B, 1, E]
    t0e = time_mlp(np.zeros_like(tp))[:, None, :]
    mask = np.asarray(mask_pos, bool)             # [B, N, 1]
    time_emb = np.where(mask, te, t0e).astype(np.float32)  # [B, N, E]

    rest = time_emb.copy()
    rest[..., INTER:] += ang_f.astype(np.float32)
    return rest.astype(np.float32)


def _split_f16(x, n):
    """n-component fp16 split: sum of components == x to n*11 mantissa bits."""
    comps = []
    r = np.asarray(x, np.float64)
    for _ in range(n):
        c = r.astype(np.float16).astype(np.float64)
        comps.append(c)
        r = r - c
    return comps


def _d2_gemm_operands(pos_b, pad_b, r0):
    """Host-prep the 24-row fp16 operands of the d^2 matmul.

    d^2(q, j) = |a|^2 - 2 a.b + |b|^2 with a = pos[q], b = pos[j]; every
    product of 3-component fp16 splits that matters to f32 precision gets
    its own contraction row (6 cross combos per coord + 3 |b|^2 comps
    against ones + 3 |a|^2 comps against ones)."""
    a = np.asarray(pos_b, np.float64)[r0 : r0 + RPC]     # queries [192, 3]
    bk = np.asarray(pos_b, np.float64).T.copy()          # keys    [3, N]
    if pad_b.any():
        bk[:, pad_b] = 1.0e6
    mk = np.zeros((NROWS_A, N), np.float16)
    sq = np.zeros((NROWS_A, RPC), np.float16)
    r = 0
    for c in range(3):
        bh, bm, bl = _split_f16(bk[c], 3)
        ah, am, al = _split_f16(a[:, c], 3)
        for (bc, ac) in ((bh, ah), (bm, ah), (bh, am),
                         (bl, ah), (bh, al), (bm, am)):
            mk[r] = bc.astype(np.float16)
            sq[r] = (-2.0 * ac).astype(np.float16)
            r += 1
    bsq = np.sum(bk * bk, axis=0)
    for comp in _split_f16(bsq, 3):
        mk[r] = comp.astype(np.float16)
        sq[r] = np.float16(1.0)
        r += 1
    asq = np.sum(a * a, axis=1)
    for comp in _split_f16(asq, 3):
        mk[r] = np.float16(1.0)
        sq[r] = comp.astype(np.float16)
        r += 1
    assert r == NROWS_A
    return mk, sq


def _prep_in_maps(pos, angle, padding_mask, mask_pos, time_pos,
                  means, stds, fp_w1, fp_w2, ang_w1, ang_w2,
                  t_w1, t_b1, t_w2, t_b2):
    pos = np.asarray(pos, np.float32)
    pad = np.asarray(padding_mask, bool)

    s = (np.abs(np.asarray(stds, np.float32)) + np.float32(0.01)).astype(np.float32)
    m = np.asarray(means, np.float32)
    inv_s = (np.float32(1.0) / s).astype(np.float32)
    # Derivative_Erf(x) with x = (d - m)/(s*sqrt(2))
    esc_v = (inv_s / np.float32(math.sqrt(2.0))).astype(np.float32)
    ebi_v = (-m * esc_v).astype(np.float32)
    postc_v = (
        np.float32(DERF_INV) / (np.float32(math.sqrt(2.0 * PI_REF)) * s)
    ).astype(np.float32)

    rest = _host_tails(
        angle, mask_pos, time_pos, ang_w1, ang_w2, t_w1, t_b1, t_w2, t_b2
    )

    w1_v = np.asarray(fp_w1, np.float16)
    w2_v = np.asarray(fp_w2, np.float16)

    in_maps = []
    for c in range(NCORES):
        b = c // (NCORES // B)
        r0 = (c % (NCORES // B)) * RPC
        mk, sq = _d2_gemm_operands(pos[b], pad[b], r0)
        in_maps.append(
            {
                "mkeys": mk,
                "squery": sq,
                "esc": esc_v.reshape(K, 1),
                "ebi": ebi_v.reshape(K, 1),
                "postc": postc_v.reshape(K, 1),
                "w1": w1_v,
                "w2": w2_v,
                "ident": np.eye(128, dtype=np.float32),
                "rest": np.ascontiguousarray(rest[b, r0 : r0 + RPC, :], np.float32),
            }
        )
    return in_maps


def kernel(pos, angle, node_type_edge, padding_mask, mask_aa, mask_pos, time_pos,
           means, stds, fp_w1, fp_w2, ang_w1, ang_w2, t_w1, t_b1, t_w2, t_b2):
    from concourse.bass_utils import run_bass_kernel_spmd

    key = "nc_v3"
    if key not in _COMPILED:
        _COMPILED[key] = _build_nc()
    nc = _COMPILED[key]

    in_maps = _prep_in_maps(
        pos, angle, padding_mask, mask_pos, time_pos, means, stds,
        fp_w1, fp_w2, ang_w1, ang_w2, t_w1, t_b1, t_w2, t_b2,
    )
    res = run_bass_kernel_spmd(nc, in_maps, core_ids=list(range(NCORES)))
    outs = [np.asarray(res.results[c]["out"], np.float32) for c in range(NCORES)]
    full = np.concatenate(outs, axis=0).reshape(B, N, E)
    return full


# revision 19
# speedup vs baseline: 1.1561x; 1.1561x over previous
"""Trainium2 Bass kernel for nn_Node3DEmbeddingv2 (gnn_message_passing).

Strategy (8 NeuronCores, SPMD, data-parallel over flattened (batch, query-row)):
  - 1536 query rows split into 8 x 192 (4 cores per batch).
  - Phase A (per 96-row strip): d^2 = |pi|^2 - 2 pi.pj + |pj|^2 via one fp16
    matmul against host-prepped 3-component fp16 splits of pos (24
    contraction rows, exact to f32), ACT Relu(x - 5e-4) to clamp the
    cancellation noise on the diagonal to exactly 0, ACT Sqrt, then an exact
    3-way fp16 split of d (33 mantissa bits).
  - Phase B (per 24-row block): flatten the fp16 d-components into a
    [3, 24*768] moving tile (partition trio 0-2), broadcast each row's 768
    distances across all 128 gaussian-channel partitions with a [3,128]-ones
    fp16 matmul per 512-col PSUM window (one stationary, reloaded cheaply /
    elided by ldw-opt). One ScalarE op per [128,1536] PSUM unit computes the
    Gaussian:
      Derivative_Erf(scale_k * d + bias_k) = 2/sqrt(pi) * exp(-((d-m_k)/s_k)^2/2)
    writing fp16. The key-axis sum runs as an in-place halving add-tree on
    DVE (fp16 tensor_tensor = 2x perf mode; tensor_reduce is 1x-capped) down
    to width 6, then one tiny f32 tensor_reduce into S.
  - Phase C: channel constants on the summed [128,192] tensor, fp16
    feature_proj MLP (gelu between two matmuls), DMA-transpose (fp16) back
    to row-major, add the host-computed angle/time tail, DMA out [192,512].
  - Host (numpy, negligible): angle MLP, sinusoidal time embedding MLP,
    masking, per-core input prep; all heavy compute is on-device.
"""

import math

import numpy as np

# Problem constants (hardcoded per the task contract).
B, N, K, E = 2, 768, 128, 512
INTER = E // 2
NCORES = 8
RPC = (B * N) // NCORES  # 192 rows per core
PI_REF = 3.14159         # matches reference's gaussian constant

NBLOCK = 8               # 24-row phase-B blocks per core
BR = 24                  # rows per block
NROWS_A = 24             # contraction rows of the d^2 matmul
D2_SHIFT = 5.0e-4        # relu clamp: zeroes |d| < 0.022 (true data min ~0.5)

# Derivative_Erf table semantics: d/dx erf(x) = 2/sqrt(pi) * exp(-x^2).
# DERF_INV converts the table output back to exp(-x^2).
DERF_INV = math.sqrt(math.pi) / 2.0

USE_LDW_OPT = False  # walrus rejects ldw-opt for these ldweights forms

_COMPILED = {}


def _enable_ldw_opt():
    """Flip walrus's redundant-LDWEIGHTS elimination on: the 288 broadcast
    matmuls reuse one stationary [3,128] ones matrix and the per-matmul
    reload serializes ~124ns each on the PE. Only safe with zero f32
    matmuls in the module (this kernel is all-fp16). Correctness is
    re-verified end-to-end against the oracle after any flag change."""
    from concourse import bass_utils

    if getattr(bass_utils, "_ldw_opt_patched", False):
        return
    orig_run = bass_utils.run_command

    def run_patched(argv, **kw):
        argv = [
            a.replace("--enable-ldw-opt=false", "--enable-ldw-opt=true")
            if isinstance(a, str) else a
            for a in argv
        ]
        return orig_run(argv, **kw)

    bass_utils.run_command = run_patched
    bass_utils._ldw_opt_patched = True


def _build_nc():
    import concourse.bass as bass
    import concourse.bacc as bacc
    from concourse import mybir
    from concourse.tile import TileContext

    if USE_LDW_OPT:
        _enable_ldw_opt()

    f32 = mybir.dt.float32
    f16 = mybir.dt.float16
    AF = mybir.ActivationFunctionType

    nc = bacc.Bacc("TRN2", target_bir_lowering=False)

    mkeys = nc.dram_tensor("mkeys", [NROWS_A, N], f16, kind="ExternalInput")
    squery = nc.dram_tensor("squery", [NROWS_A, RPC], f16, kind="ExternalInput")
    esc = nc.dram_tensor("esc", [K, 1], f32, kind="ExternalInput")
    ebi = nc.dram_tensor("ebi", [K, 1], f32, kind="ExternalInput")
    postc = nc.dram_tensor("postc", [K, 1], f32, kind="ExternalInput")
    w1 = nc.dram_tensor("w1", [K, K], f16, kind="ExternalInput")
    w2 = nc.dram_tensor("w2", [K, INTER], f16, kind="ExternalInput")
    ident = nc.dram_tensor("ident", [128, 128], f32, kind="ExternalInput")
    rest = nc.dram_tensor("rest", [RPC, E], f32, kind="ExternalInput")
    out = nc.dram_tensor("out", [RPC, E], f32, kind="ExternalOutput")

    with TileContext(nc) as tc:
        with nc.allow_low_precision(reason="fp16 gaussian accumulate, verified vs oracle"), \
             tc.tile_pool(name="sb", bufs=1) as sb:
            # ---- constant loads (spread over queues; phase A only needs
            # mkeys/squery, which go first on their own queues) ----
            mk_sb = sb.tile([NROWS_A, N], f16, tag="mk_sb")
            nc.sync.dma_start(out=mk_sb, in_=mkeys[:, :])
            sq_sb = sb.tile([NROWS_A, RPC], f16, tag="sq_sb")
            nc.scalar.dma_start(out=sq_sb, in_=squery[:, :])
            esc_sb = sb.tile([K, 1], f32, tag="esc_sb")
            nc.scalar.dma_start(out=esc_sb, in_=esc[:, :])
            ebi_sb = sb.tile([K, 1], f32, tag="ebi_sb")
            nc.scalar.dma_start(out=ebi_sb, in_=ebi[:, :])
            postc_sb = sb.tile([K, 1], f32, tag="postc_sb")
            nc.sync.dma_start(out=postc_sb, in_=postc[:, :])
            w1_sb = sb.tile([K, K], f16, tag="w1_sb")
            nc.sync.dma_start(out=w1_sb, in_=w1[:, :])
            w2_sb = sb.tile([K, INTER], f16, tag="w2_sb")
            nc.sync.dma_start(out=w2_sb, in_=w2[:, :])
            id_sb = sb.tile([128, 128], f32, tag="id_sb")
            nc.sync.dma_start(out=id_sb, in_=ident[:, :])
            rest_sb = sb.tile([96, 2, E], f32, tag="rest_sb")
            nc.sync.dma_start(
                out=rest_sb, in_=rest.rearrange("(s p) e -> p s e", s=2)
            )
            ones3 = sb.tile([3, 128], f16, tag="ones3")
            nc.vector.memset(ones3, 1.0)
            shift_sb = sb.tile([96, 1], f32, tag="shift_sb")
            nc.vector.memset(shift_sb, -D2_SHIFT)

            S = sb.tile([K, RPC], f32, tag="S")

            # Collapse the input-DMA queue semaphores into one point.
            tc.strict_bb_all_engine_barrier()

            dcomps = {}

            def phase_a(st):
                # d^2 matmul + relu/sqrt + fp16 splits for one 96-row strip
                d2p = psA.tile([96, N], f32, tag="d2p", bufs=1)
                cols = slice(96 * st, 96 * (st + 1))
                nc.tensor.matmul(
                    d2p[:, 0:512], sq_sb[:, cols], mk_sb[:, 0:512],
                    start=True, stop=True,
                )
                nc.tensor.matmul(
                    d2p[:, 512:N], sq_sb[:, cols], mk_sb[:, 512:N],
                    start=True, stop=True,
                )
                dr = sb.tile([96, N], f32, tag="dr", bufs=2)
                nc.scalar.activation(dr, d2p, AF.Relu, bias=shift_sb)
                d32 = sb.tile([96, N], f32, tag="d32", bufs=2)
                nc.scalar.sqrt(d32, dr)
                dc = sb.tile([96, 3, N], f16, tag="dcomp", bufs=2)
                nc.vector.tensor_copy(dc[:, 0, :], d32)
                r1 = sb.tile([96, N], f32, tag="r1", bufs=2)
                nc.vector.tensor_sub(r1, d32, dc[:, 0, :])
                nc.vector.tensor_copy(dc[:, 1, :], r1)
                r2 = sb.tile([96, N], f32, tag="r2", bufs=2)
                nc.vector.tensor_sub(r2, r1, dc[:, 1, :])
                nc.vector.tensor_copy(dc[:, 2, :], r2)
                dcomps[st] = dc

            def bcast_matmul(out_ap, rhs, ldw):
                # nc.tensor.matmul with an explicit ldweights flag: all
                # broadcast matmuls share the ones3 stationary, so runs after
                # the first skip the per-matmul LDWEIGHTS reload (ldw=False).
                ifmap_ap = nc.tensor.lower_ap(rhs.opt({0}), opt=False)
                weights_ap = nc.tensor.lower_ap(
                    ones3.opt({0}), opt=False, for_matmul_weights=True
                )
                out_l = nc.tensor.lower_ap(out_ap)
                nc.tensor.add_instruction(
                    mybir.InstMatmult(
                        name=nc.get_next_instruction_name(),
                        replication_resolution=0,
                        replication_shift_amnt=0,
                        replication_num_rows=0,
                        start_tensor_calc=True,
                        stop_tensor_calc=True,
                        ins=[ifmap_ap, weights_ap],
                        outs=[out_l],
                        tile_position=(0, 0),
                        tile_size=(32, 128),
                        ldweights=ldw,
                    )
                )

            def phase_b(b):
                # broadcast + gaussian + key-sum for one 24-row block
                dc = dcomps[b // 4]
                p0 = BR * (b % 4)
                mfl = sb.tile([3, BR * N], f16, tag="mflat", bufs=2)
                for comp in range(3):
                    nc.gpsimd.dma_start(
                        out=mfl[comp : comp + 1, :].rearrange(
                            "p (i j) -> p i j", i=BR
                        ),
                        in_=dc[p0 : p0 + BR, comp, :],
                    )
                gsc = sb.tile([K, BR, N], f16, tag="gsc", bufs=2)
                for u in range(12):
                    pu = psB.tile([K, 1536], f32, tag="pu", bufs=2)
                    for w in range(3):
                        lo = 1536 * u + 512 * w
                        # reload weights on the first matmul after any
                        # other-stationary matmul ran on the PE (phase A of
                        # strip 1 is emitted between blocks 1 and 2)
                        ldw = u == 0 and w == 0 and b in (0, 2)
                        bcast_matmul(
                            pu[:, 512 * w : 512 * w + 512],
                            mfl[:, lo : lo + 512],
                            ldw,
                        )
                    nc.scalar.activation(
                        out=gsc[:, 2 * u : 2 * u + 2, :],
                        in_=pu,
                        func=AF.Derivative_Erf,
                        bias=ebi_sb,
                        scale=esc_sb,
                    )
                # in-place fp16 halving add-tree over the key axis (runs
                # during the NEXT block's gaussians — other gsc buffer)
                wdt = N // 2
                while wdt >= 6:
                    nc.vector.tensor_add(
                        gsc[:, :, 0:wdt], gsc[:, :, 0:wdt],
                        gsc[:, :, wdt : 2 * wdt],
                    )
                    wdt //= 2
                nc.vector.tensor_reduce(
                    out=S[:, BR * b : BR * (b + 1)],
                    in_=gsc[:, :, 0:6],
                    axis=mybir.AxisListType.X,
                    op=mybir.AluOpType.add,
                )

            with tc.tile_pool(name="psA", bufs=1, space="PSUM") as psA, \
                 tc.tile_pool(name="psB", bufs=1, space="PSUM") as psB:
                phase_a(0)
                for b in range(NBLOCK):
                    if b == 2:
                        phase_a(1)
                    phase_b(b)

            # ---- phase C: channel constants + feature_proj MLP + output ----
            with tc.tile_pool(name="psC", bufs=1, space="PSUM") as psC:
                for st in range(2):
                    rows = slice(96 * st, 96 * (st + 1))
                    nc.vector.tensor_scalar_mul(S[:, rows], S[:, rows], postc_sb)
                    s16 = sb.tile([K, 96], f16, tag="s16", bufs=2)
                    nc.vector.tensor_copy(s16, S[:, rows])
                    psum_h = psC.tile([K, 96], f32, tag="mlp_h", bufs=2)
                    nc.tensor.matmul(psum_h, w1_sb, s16, start=True, stop=True)
                    h16 = sb.tile([K, 96], f16, tag="h16", bufs=2)
                    nc.scalar.activation(h16, psum_h, AF.Gelu)
                    o_sb = sb.tile([K, 2, 96], f32, tag="o_sb", bufs=2)
                    for e in range(2):
                        psum_o = psC.tile([K, 96], f32, tag="mlp_o", bufs=2)
                        nc.tensor.matmul(
                            psum_o, w2_sb[:, 128 * e : 128 * (e + 1)], h16,
                            start=True, stop=True,
                        )
                        nc.scalar.copy(o_sb[:, e, :], psum_o)
                    for e in range(2):
                        psum_t = psC.tile([96, 128], f32, tag="tr", bufs=2)
                        nc.tensor.transpose(psum_t, o_sb[:, e, :], id_sb)
                        nc.vector.tensor_add(
                            rest_sb[:, st, 128 * e : 128 * (e + 1)],
                            rest_sb[:, st, 128 * e : 128 * (e + 1)],
                            psum_t,
                        )
                    nc.sync.dma_start(out=out[rows, :], in_=rest_sb[:, st, :])

    nc.compile()
    return nc


# ---------------- host-side prep (numpy) ----------------

def _erf_np(x):
    try:
        from scipy.special import erf
        return erf(x).astype(np.float32)
    except ImportError:
        f = np.frompyfunc(math.erf, 1, 1)
        return f(x.astype(np.float64)).astype(np.float32)


def _gelu_np(x):
    x = x.astype(np.float32)
    return (x * 0.5 * (1.0 + _erf_np(x / np.float32(math.sqrt(2.0))))).astype(
        np.float32
    )


def _silu_np(x):
    x = x.astype(np.float32)
    return (x / (1.0 + np.exp(-x))).astype(np.float32)


def _timestep_emb_np(t, dim):
    half = dim // 2
    freqs = np.exp(
        -np.log(10000.0) * np.arange(half, dtype=np.float32) / np.float32(half)
    ).astype(np.float32)
    a = t.astype(np.float32)[:, None] * freqs[None, :]
    return np.concatenate([np.sin(a), np.cos(a)], axis=-1).astype(np.float32)


def _host_tails(angle, mask_pos, time_pos, ang_w1, ang_w2, t_w1, t_b1, t_w2, t_b2):
    """rest[b, n, :] with rest[..., :INTER] = time_emb[..., :INTER] and
    rest[..., INTER:] = ang_f + time_emb[..., INTER:]."""
    angle = np.asarray(angle, np.float32)
    ang = np.where(np.isposinf(angle), np.float32(0.0), angle).astype(np.float32)
    ang_f = _gelu_np(ang @ np.asarray(ang_w1, np.float32)) @ np.asarray(
        ang_w2, np.float32
    )  # [B, N, INTER]

    def time_mlp(t):
        e = _timestep_emb_np(t, E)
        h = _silu_np(e @ np.asarray(t_w1, np.float32) + np.asarray(t_b1, np.float32))
        return (h @ np.asarray(t_w2, np.float32) + np.asarray(t_b2, np.float32)).astype(
            np.float32
        )

    tp = np.asarray(time_pos)
    te = time_mlp(tp)[:, None, :]                 # [B, 1, E]
    t0e = time_mlp(np.zeros_like(tp))[:, None, :]
    mask = np.asarray(mask_pos, bool)             # [B, N, 1]
    time_emb = np.where(mask, te, t0e).astype(np.float32)  # [B, N, E]

    rest = time_emb.copy()
    rest[..., INTER:] += ang_f.astype(np.float32)
    return rest.astype(np.float32)


def _split_f16(x, n):
    """n-component fp16 split: sum of components == x to n*11 mantissa bits."""
    comps = []
    r = np.asarray(x, np.float64)
    for _ in range(n):
        c = r.astype(np.float16).astype(np.float64)
        comps.append(c)
        r = r - c
    return comps


def _d2_gemm_operands(pos_b, pad_b, r0):
    """Host-prep the 24-row fp16 operands of the d^2 matmul.

    d^2(q, j) = |a|^2 - 2 a.b + |b|^2 with a = pos[q], b = pos[j]; every
    product of 3-component fp16 splits that matters to f32 precision gets
    its own contraction row (6 cross combos per coord + 3 |b|^2 comps
    against ones + 3 |a|^2 comps against ones)."""
    a = np.asarray(pos_b, np.float64)[r0 : r0 + RPC]     # queries [192, 3]
    bk = np.asarray(pos_b, np.float64).T.copy()          # keys    [3, N]
    if pad_b.any():
        bk[:, pad_b] = 1.0e6
    mk = np.zeros((NROWS_A, N), np.float16)
    sq = np.zeros((NROWS_A, RPC), np.float16)
    r = 0
    for c in range(3):
        bh, bm, bl = _split_f16(bk[c], 3)
        ah, am, al = _split_f16(a[:, c], 3)
        for (bc, ac) in ((bh, ah), (bm, ah), (bh, am),
                         (bl, ah), (bh, al), (bm, am)):
            mk[r] = bc.astype(np.float16)
            sq[r] = (-2.0 * ac).astype(np.float16)
            r += 1
    bsq = np.sum(bk * bk, axis=0)
    for comp in _split_f16(bsq, 3):
        mk[r] = comp.astype(np.float16)
        sq[r] = np.float16(1.0)
        r += 1
    asq = np.sum(a * a, axis=1)
    for comp in _split_f16(asq, 3):
        mk[r] = np.float16(1.0)
        sq[r] = comp.astype(np.float16)
        r += 1
    assert r == NROWS_A
    return mk, sq


def _prep_in_maps(pos, angle, padding_mask, mask_pos, time_pos,
                  means, stds, fp_w1, fp_w2, ang_w1, ang_w2,
                  t_w1, t_b1, t_w2, t_b2):
    pos = np.asarray(pos, np.float32)
    pad = np.asarray(padding_mask, bool)

    s = (np.abs(np.asarray(stds, np.float32)) + np.float32(0.01)).astype(np.float32)
    m = np.asarray(means, np.float32)
    inv_s = (np.float32(1.0) / s).astype(np.float32)
    # Derivative_Erf(x) with x = (d - m)/(s*sqrt(2))
    esc_v = (inv_s / np.float32(math.sqrt(2.0))).astype(np.float32)
    ebi_v = (-m * esc_v).astype(np.float32)
    postc_v = (
        np.float32(DERF_INV) / (np.float32(math.sqrt(2.0 * PI_REF)) * s)
    ).astype(np.float32)

    rest = _host_tails(
        angle, mask_pos, time_pos, ang_w1, ang_w2, t_w1, t_b1, t_w2, t_b2
    )

    w1_v = np.asarray(fp_w1, np.float16)
    w2_v = np.asarray(fp_w2, np.float16)

    in_maps = []
    for c in range(NCORES):
        b = c // (NCORES // B)
        r0 = (c % (NCORES // B)) * RPC
        mk, sq = _d2_gemm_operands(pos[b], pad[b], r0)
        in_maps.append(
            {
                "mkeys": mk,
                "squery": sq,
                "esc": esc_v.reshape(K, 1),
                "ebi": ebi_v.reshape(K, 1),
                "postc": postc_v.reshape(K, 1),
                "w1": w1_v,
                "w2": w2_v,
                "ident": np.eye(128, dtype=np.float32),
                "rest": np.ascontiguousarray(rest[b, r0 : r0 + RPC, :], np.float32),
            }
        )
    return in_maps


def kernel(pos, angle, node_type_edge, padding_mask, mask_aa, mask_pos, time_pos,
           means, stds, fp_w1, fp_w2, ang_w1, ang_w2, t_w1, t_b1, t_w2, t_b2):
    from concourse.bass_utils import run_bass_kernel_spmd

    key = "nc_v3"
    if key not in _COMPILED:
        _COMPILED[key] = _build_nc()
    nc = _COMPILED[key]

    in_maps = _prep_in_maps(
        pos, angle, padding_mask, mask_pos, time_pos, means, stds,
        fp_w1, fp_w2, ang_w1, ang_w2, t_w1, t_b1, t_w2, t_b2,
    )
    res = run_bass_kernel_spmd(nc, in_maps, core_ids=list(range(NCORES)))
    outs = [np.asarray(res.results[c]["out"], np.float32) for c in range(NCORES)]
    full = np.concatenate(outs, axis=0).reshape(B, N, E)
    return full


# revision 22
# speedup vs baseline: 1.1861x; 1.0260x over previous
"""Trainium2 Bass kernel for nn_Node3DEmbeddingv2 (gnn_message_passing).

Strategy (8 NeuronCores, SPMD, data-parallel over flattened (batch, query-row)):
  - 1536 query rows split into 8 x 192 (4 cores per batch).
  - Phase A (per 96-row strip): d^2 = |pi|^2 - 2 pi.pj + |pj|^2 via one fp16
    matmul against host-prepped 3-component fp16 splits of pos (24
    contraction rows, exact to f32), ACT Relu(x - 5e-4) to clamp the
    cancellation noise on the diagonal to exactly 0, ACT Sqrt, then an exact
    3-way fp16 split of d (33 mantissa bits).
  - Phase B (per 24-row block): flatten the fp16 d-components into a
    [3, 24*768] moving tile (partition trio 0-2), broadcast each row's 768
    distances across all 128 gaussian-channel partitions with a [3,128]-ones
    fp16 matmul per 512-col PSUM window (one stationary, reloaded cheaply /
    elided by ldw-opt). One ScalarE op per [128,1536] PSUM unit computes the
    Gaussian:
      Derivative_Erf(scale_k * d + bias_k) = 2/sqrt(pi) * exp(-((d-m_k)/s_k)^2/2)
    writing fp16. The key-axis sum runs as an in-place halving add-tree on
    DVE (fp16 tensor_tensor = 2x perf mode; tensor_reduce is 1x-capped) down
    to width 6, then one tiny f32 tensor_reduce into S.
  - Phase C: channel constants on the summed [128,192] tensor, fp16
    feature_proj MLP (gelu between two matmuls), DMA-transpose (fp16) back
    to row-major, add the host-computed angle/time tail, DMA out [192,512].
  - Host (numpy, negligible): angle MLP, sinusoidal time embedding MLP,
    masking, per-core input prep; all heavy compute is on-device.
"""

import math

import numpy as np

# Problem constants (hardcoded per the task contract).
B, N, K, E = 2, 768, 128, 512
INTER = E // 2
NCORES = 8
RPC = (B * N) // NCORES  # 192 rows per core
PI_REF = 3.14159         # matches reference's gaussian constant

NBLOCK = 8               # 24-row phase-B blocks per core
BR = 24                  # rows per block
NROWS_A = 24             # contraction rows of the d^2 matmul
D2_SHIFT = 5.0e-4        # relu clamp: zeroes |d| < 0.022 (true data min ~0.5)

# Derivative_Erf table semantics: d/dx erf(x) = 2/sqrt(pi) * exp(-x^2).
# DERF_INV converts the table output back to exp(-x^2).
DERF_INV = math.sqrt(math.pi) / 2.0

USE_LDW_OPT = False  # walrus rejects ldw-opt for these ldweights forms

_COMPILED = {}


def _enable_ldw_opt():
    """Flip walrus's redundant-LDWEIGHTS elimination on: the 288 broadcast
    matmuls reuse one stationary [3,128] ones matrix and the per-matmul
    reload serializes ~124ns each on the PE. Only safe with zero f32
    matmuls in the module (this kernel is all-fp16). Correctness is
    re-verified end-to-end against the oracle after any flag change."""
    from concourse import bass_utils

    if getattr(bass_utils, "_ldw_opt_patched", False):
        return
    orig_run = bass_utils.run_command

    def run_patched(argv, **kw):
        argv = [
            a.replace("--enable-ldw-opt=false", "--enable-ldw-opt=true")
            if isinstance(a, str) else a
            for a in argv
        ]
        return orig_run(argv, **kw)

    bass_utils.run_command = run_patched
    bass_utils._ldw_opt_patched = True


def _build_nc():
    import concourse.bass as bass
    import concourse.bacc as bacc
    from concourse import mybir
    from concourse.tile import TileContext

    if USE_LDW_OPT:
        _enable_ldw_opt()

    f32 = mybir.dt.float32
    f16 = mybir.dt.float16
    AF = mybir.ActivationFunctionType

    nc = bacc.Bacc("TRN2", target_bir_lowering=False)

    mkeys = nc.dram_tensor("mkeys", [NROWS_A, N], f16, kind="ExternalInput")
    squery = nc.dram_tensor("squery", [NROWS_A, RPC], f16, kind="ExternalInput")
    esc = nc.dram_tensor("esc", [K, 1], f32, kind="ExternalInput")
    ebi = nc.dram_tensor("ebi", [K, 1], f32, kind="ExternalInput")
    postc = nc.dram_tensor("postc", [K, 1], f32, kind="ExternalInput")
    w1 = nc.dram_tensor("w1", [K, K], f16, kind="ExternalInput")
    w2 = nc.dram_tensor("w2", [K, INTER], f16, kind="ExternalInput")
    ident = nc.dram_tensor("ident", [128, 128], f32, kind="ExternalInput")
    rest = nc.dram_tensor("rest", [RPC, E], f32, kind="ExternalInput")
    out = nc.dram_tensor("out", [RPC, E], f32, kind="ExternalOutput")

    with TileContext(nc) as tc:
        with nc.allow_low_precision(reason="fp16 gaussian accumulate, verified vs oracle"), \
             tc.tile_pool(name="sb", bufs=1) as sb:
            # ---- constant loads (spread over queues; phase A only needs
            # mkeys/squery, which go first on their own queues) ----
            mk_sb = sb.tile([NROWS_A, N], f16, tag="mk_sb")
            nc.sync.dma_start(out=mk_sb, in_=mkeys[:, :])
            sq_sb = sb.tile([NROWS_A, RPC], f16, tag="sq_sb")
            nc.scalar.dma_start(out=sq_sb, in_=squery[:, :])
            esc_sb = sb.tile([K, 1], f32, tag="esc_sb")
            nc.scalar.dma_start(out=esc_sb, in_=esc[:, :])
            ebi_sb = sb.tile([K, 1], f32, tag="ebi_sb")
            nc.scalar.dma_start(out=ebi_sb, in_=ebi[:, :])
            postc_sb = sb.tile([K, 1], f32, tag="postc_sb")
            nc.sync.dma_start(out=postc_sb, in_=postc[:, :])
            w1_sb = sb.tile([K, K], f16, tag="w1_sb")
            nc.sync.dma_start(out=w1_sb, in_=w1[:, :])
            w2_sb = sb.tile([K, INTER], f16, tag="w2_sb")
            nc.sync.dma_start(out=w2_sb, in_=w2[:, :])
            ones3 = sb.tile([3, 128], f16, tag="ones3")
            nc.vector.memset(ones3, 1.0)
            shift_sb = sb.tile([96, 1], f32, tag="shift_sb")
            nc.vector.memset(shift_sb, -D2_SHIFT)

            S = sb.tile([K, RPC], f32, tag="S")

            # Collapse the input-DMA queue semaphores into one point.
            tc.strict_bb_all_engine_barrier()

            # phase-C-only loads go after the barrier: they overlap phase B
            # on the otherwise-idle sync queue instead of delaying phase A
            id_sb = sb.tile([128, 128], f32, tag="id_sb")
            nc.sync.dma_start(out=id_sb, in_=ident[:, :])
            rest_sb = sb.tile([96, 2, E], f32, tag="rest_sb")
            nc.sync.dma_start(
                out=rest_sb, in_=rest.rearrange("(s p) e -> p s e", s=2)
            )

            dcomps = {}

            def phase_a(st):
                # d^2 matmul + relu/sqrt + fp16 splits for one 96-row strip
                d2p = psA.tile([96, N], f32, tag="d2p", bufs=1)
                cols = slice(96 * st, 96 * (st + 1))
                nc.tensor.matmul(
                    d2p[:, 0:512], sq_sb[:, cols], mk_sb[:, 0:512],
                    start=True, stop=True,
                )
                nc.tensor.matmul(
                    d2p[:, 512:N], sq_sb[:, cols], mk_sb[:, 512:N],
                    start=True, stop=True,
                )
                dr = sb.tile([96, N], f32, tag="dr", bufs=2)
                nc.scalar.activation(dr, d2p, AF.Relu, bias=shift_sb)
                d32 = sb.tile([96, N], f32, tag="d32", bufs=2)
                nc.scalar.sqrt(d32, dr)
                dc = sb.tile([96, 3, N], f16, tag="dcomp", bufs=2)
                nc.vector.tensor_copy(dc[:, 0, :], d32)
                r1 = sb.tile([96, N], f32, tag="r1", bufs=2)
                nc.vector.tensor_sub(r1, d32, dc[:, 0, :])
                nc.vector.tensor_copy(dc[:, 1, :], r1)
                r2 = sb.tile([96, N], f32, tag="r2", bufs=2)
                nc.vector.tensor_sub(r2, r1, dc[:, 1, :])
                nc.vector.tensor_copy(dc[:, 2, :], r2)
                dcomps[st] = dc

            def bcast_matmul(out_ap, rhs, ldw):
                # nc.tensor.matmul with an explicit ldweights flag: all
                # broadcast matmuls share the ones3 stationary, so runs after
                # the first skip the per-matmul LDWEIGHTS reload (ldw=False).
                ifmap_ap = nc.tensor.lower_ap(rhs.opt({0}), opt=False)
                weights_ap = nc.tensor.lower_ap(
                    ones3.opt({0}), opt=False, for_matmul_weights=True
                )
                out_l = nc.tensor.lower_ap(out_ap)
                nc.tensor.add_instruction(
                    mybir.InstMatmult(
                        name=nc.get_next_instruction_name(),
                        replication_resolution=0,
                        replication_shift_amnt=0,
                        replication_num_rows=0,
                        start_tensor_calc=True,
                        stop_tensor_calc=True,
                        ins=[ifmap_ap, weights_ap],
                        outs=[out_l],
                        tile_position=(0, 0),
                        tile_size=(32, 128),
                        ldweights=ldw,
                    )
                )

            def phase_b(row0, nrows, reload_w):
                # broadcast + gaussian + key-sum for one row block
                dc = dcomps[row0 // 96]
                p0 = row0 % 96
                mfl = sb.tile([3, BR * N], f16, tag="mflat", bufs=2)
                for comp in range(3):
                    nc.gpsimd.dma_start(
                        out=mfl[comp : comp + 1, 0 : nrows * N].rearrange(
                            "p (i j) -> p i j", i=nrows
                        ),
                        in_=dc[p0 : p0 + nrows, comp, :],
                    )
                gsc = sb.tile([K, BR, N], f16, tag="gsc", bufs=2)
                for u in range(nrows // 2):
                    pu = psB.tile([K, 1536], f32, tag="pu", bufs=2)
                    for w in range(3):
                        lo = 1536 * u + 512 * w
                        # reload weights on the first matmul after any
                        # other-stationary matmul ran on the PE (phase A of
                        # strip 1 is emitted between blocks 1 and 2)
                        ldw = u == 0 and w == 0 and reload_w
                        bcast_matmul(
                            pu[:, 512 * w : 512 * w + 512],
                            mfl[:, lo : lo + 512],
                            ldw,
                        )
                    nc.scalar.activation(
                        out=gsc[:, 2 * u : 2 * u + 2, :],
                        in_=pu,
                        func=AF.Derivative_Erf,
                        bias=ebi_sb,
                        scale=esc_sb,
                    )
                # in-place fp16 halving add-tree over the key axis (runs
                # during the NEXT block's gaussians — other gsc buffer)
                wdt = N // 2
                while wdt >= 6:
                    nc.vector.tensor_add(
                        gsc[:, 0:nrows, 0:wdt], gsc[:, 0:nrows, 0:wdt],
                        gsc[:, 0:nrows, wdt : 2 * wdt],
                    )
                    wdt //= 2
                nc.vector.tensor_reduce(
                    out=S[:, row0 : row0 + nrows],
                    in_=gsc[:, 0:nrows, 0:6],
                    axis=mybir.AxisListType.X,
                    op=mybir.AluOpType.add,
                )

            # last 24 rows split into two 12-row blocks: the final serial
            # add-tree (pure tail latency) halves
            blocks = [(24 * b, 24) for b in range(7)] + [(168, 12), (180, 12)]
            with tc.tile_pool(name="psA", bufs=1, space="PSUM") as psA, \
                 tc.tile_pool(name="psB", bufs=1, space="PSUM") as psB:
                phase_a(0)
                for bi, (row0, nrows) in enumerate(blocks):
                    if bi == 2:
                        phase_a(1)
                    phase_b(row0, nrows, reload_w=bi in (0, 2))

            # ---- phase C: channel constants + feature_proj MLP + output ----
            with tc.tile_pool(name="psC", bufs=1, space="PSUM") as psC:
                for st in range(2):
                    rows = slice(96 * st, 96 * (st + 1))
                    nc.vector.tensor_scalar_mul(S[:, rows], S[:, rows], postc_sb)
                    s16 = sb.tile([K, 96], f16, tag="s16", bufs=2)
                    nc.vector.tensor_copy(s16, S[:, rows])
                    psum_h = psC.tile([K, 96], f32, tag="mlp_h", bufs=2)
                    nc.tensor.matmul(psum_h, w1_sb, s16, start=True, stop=True)
                    h16 = sb.tile([K, 96], f16, tag="h16", bufs=2)
                    nc.scalar.activation(h16, psum_h, AF.Gelu)
                    o_sb = sb.tile([K, 2, 96], f32, tag="o_sb", bufs=2)
                    for e in range(2):
                        psum_o = psC.tile([K, 96], f32, tag="mlp_o", bufs=2)
                        nc.tensor.matmul(
                            psum_o, w2_sb[:, 128 * e : 128 * (e + 1)], h16,
                            start=True, stop=True,
                        )
                        nc.scalar.copy(o_sb[:, e, :], psum_o)
                    for e in range(2):
                        psum_t = psC.tile([96, 128], f32, tag="tr", bufs=2)
                        nc.tensor.transpose(psum_t, o_sb[:, e, :], id_sb)
                        nc.vector.tensor_add(
                            rest_sb[:, st, 128 * e : 128 * (e + 1)],
                            rest_sb[:, st, 128 * e : 128 * (e + 1)],
                            psum_t,
                        )
                    nc.sync.dma_start(out=out[rows, :], in_=rest_sb[:, st, :])

    nc.compile()
    return nc


# ---------------- host-side prep (numpy) ----------------

def _erf_np(x):
    try:
        from scipy.special import erf
        return erf(x).astype(np.float32)
    except ImportError:
        f = np.frompyfunc(math.erf, 1, 1)
        return f(x.astype(np.float64)).astype(np.float32)


def _gelu_np(x):
    x = x.astype(np.float32)
    return (x * 0.5 * (1.0 + _erf_np(x / np.float32(math.sqrt(2.0))))).astype(
        np.float32
    )


def _silu_np(x):
    x = x.astype(np.float32)
    return (x / (1.0 + np.exp(-x))).astype(np.float32)


def _timestep_emb_np(t, dim):
    half = dim // 2
    freqs = np.exp(
        -np.log(10000.0) * np.arange(half, dtype=np.float32) / np.float32(half)
    ).astype(np.float32)
    a = t.astype(np.float32)[:, None] * freqs[None, :]
    return np.concatenate([np.sin(a), np.cos(a)], axis=-1).astype(np.float32)


def _host_tails(angle, mask_pos, time_pos, ang_w1, ang_w2, t_w1, t_b1, t_w2, t_b2):
    """rest[b, n, :] with rest[..., :INTER] = time_emb[..., :INTER] and
    rest[..., INTER:] = ang_f + time_emb[..., INTER:]."""
    angle = np.asarray(angle, np.float32)
    ang = np.where(np.isposinf(angle), np.float32(0.0), angle).astype(np.float32)
    ang_f = _gelu_np(ang @ np.asarray(ang_w1, np.float32)) @ np.asarray(
        ang_w2, np.float32
    )  # [B, N, INTER]

    def time_mlp(t):
        e = _timestep_emb_np(t, E)
        h = _silu_np(e @ np.asarray(t_w1, np.float32) + np.asarray(t_b1, np.float32))
        return (h @ np.asarray(t_w2, np.float32) + np.asarray(t_b2, np.float32)).astype(
            np.float32
        )

    tp = np.asarray(time_pos)
    te = time_mlp(tp)[:, None, :]                 # [B, 1, E]
    t0e = time_mlp(np.zeros_like(tp))[:, None, :]
    mask = np.asarray(mask_pos, bool)             # [B, N, 1]
    time_emb = np.where(mask, te, t0e).astype(np.float32)  # [B, N, E]

    rest = time_emb.copy()
    rest[..., INTER:] += ang_f.astype(np.float32)
    return rest.astype(np.float32)


def _split_f16(x, n):
    """n-component fp16 split: sum of components == x to n*11 mantissa bits."""
    comps = []
    r = np.asarray(x, np.float64)
    for _ in range(n):
        c = r.astype(np.float16).astype(np.float64)
        comps.append(c)
        r = r - c
    return comps


def _d2_gemm_operands(pos_b, pad_b, r0):
    """Host-prep the 24-row fp16 operands of the d^2 matmul.

    d^2(q, j) = |a|^2 - 2 a.b + |b|^2 with a = pos[q], b = pos[j]; every
    product of 3-component fp16 splits that matters to f32 precision gets
    its own contraction row (6 cross combos per coord + 3 |b|^2 comps
    against ones + 3 |a|^2 comps against ones)."""
    a = np.asarray(pos_b, np.float64)[r0 : r0 + RPC]     # queries [192, 3]
    bk = np.asarray(pos_b, np.float64).T.copy()          # keys    [3, N]
    if pad_b.any():
        bk[:, pad_b] = 1.0e6
    mk = np.zeros((NROWS_A, N), np.float16)
    sq = np.zeros((NROWS_A, RPC), np.float16)
    r = 0
    for c in range(3):
        bh, bm, bl = _split_f16(bk[c], 3)
        ah, am, al = _split_f16(a[:, c], 3)
        for (bc, ac) in ((bh, ah), (bm, ah), (bh, am),
                         (bl, ah), (bh, al), (bm, am)):
            mk[r] = bc.astype(np.float16)
            sq[r] = (-2.0 * ac).astype(np.float16)
            r += 1
    bsq = np.sum(bk * bk, axis=0)
    for comp in _split_f16(bsq, 3):
        mk[r] = comp.astype(np.float16)
        sq[r] = np.float16(1.0)
        r += 1
    asq = np.sum(a * a, axis=1)
    for comp in _split_f16(asq, 3):
        mk[r] = np.float16(1.0)
        sq[r] = comp.astype(np.float16)
        r += 1
    assert r == NROWS_A
    return mk, sq


def _prep_in_maps(pos, angle, padding_mask, mask_pos, time_pos,
                  means, stds, fp_w1, fp_w2, ang_w1, ang_w2,
                  t_w1, t_b1, t_w2, t_b2):
    pos = np.asarray(pos, np.float32)
    pad = np.asarray(padding_mask, bool)

    s = (np.abs(np.asarray(stds, np.float32)) + np.float32(0.01)).astype(np.float32)
    m = np.asarray(means, np.float32)
    inv_s = (np.float32(1.0) / s).astype(np.float32)
    # Derivative_Erf(x) with x = (d - m)/(s*sqrt(2))
    esc_v = (inv_s / np.float32(math.sqrt(2.0))).astype(np.float32)
    ebi_v = (-m * esc_v).astype(np.float32)
    postc_v = (
        np.float32(DERF_INV) / (np.float32(math.sqrt(2.0 * PI_REF)) * s)
    ).astype(np.float32)

    rest = _host_tails(
        angle, mask_pos, time_pos, ang_w1, ang_w2, t_w1, t_b1, t_w2, t_b2
    )

    w1_v = np.asarray(fp_w1, np.float16)
    w2_v = np.asarray(fp_w2, np.float16)

    in_maps = []
    for c in range(NCORES):
        b = c // (NCORES // B)
        r0 = (c % (NCORES // B)) * RPC
        mk, sq = _d2_gemm_operands(pos[b], pad[b], r0)
        in_maps.append(
            {
                "mkeys": mk,
                "squery": sq,
                "esc": esc_v.reshape(K, 1),
                "ebi": ebi_v.reshape(K, 1),
                "postc": postc_v.reshape(K, 1),
                "w1": w1_v,
                "w2": w2_v,
                "ident": np.eye(128, dtype=np.float32),
                "rest": np.ascontiguousarray(rest[b, r0 : r0 + RPC, :], np.float32),
            }
        )
    return in_maps


def kernel(pos, angle, node_type_edge, padding_mask, mask_aa, mask_pos, time_pos,
           means, stds, fp_w1, fp_w2, ang_w1, ang_w2, t_w1, t_b1, t_w2, t_b2):
    from concourse.bass_utils import run_bass_kernel_spmd

    key = "nc_v3"
    if key not in _COMPILED:
        _COMPILED[key] = _build_nc()
    nc = _COMPILED[key]

    in_maps = _prep_in_maps(
        pos, angle, padding_mask, mask_pos, time_pos, means, stds,
        fp_w1, fp_w2, ang_w1, ang_w2, t_w1, t_b1, t_w2, t_b2,
    )
    res = run_bass_kernel_spmd(nc, in_maps, core_ids=list(range(NCORES)))
    outs = [np.asarray(res.results[c]["out"], np.float32) for c in range(NCORES)]
    full = np.concatenate(outs, axis=0).reshape(B, N, E)
    return full
